# revision 1
# baseline (speedup 1.0000x reference)
"""8-core Trainium2 (Bass/Tile) kernel for nn_CrossAttention.

Sharding: pure data parallelism — batch B=8, one batch element per
NeuronCore. Each core runs the full pipeline (LayerNorm on x/context,
QKV projections, 16-head attention with relative position bias,
output projection) for its element; the host gathers the 8 outputs.

Host-side prep (constant transforms of the inputs):
  - gamma folded into W{q,k,v} rows; beta folded into additive bias
    vectors beta@W{q,k,v}.
  - relative_position_bias b is shipped as exp(b)^T in fp16 (the kernel
    computes softmax numerators as exp(s)*exp(b) -- no max subtraction
    needed since scores are O(10) and everything fits fp32/fp16 ranges).
  - weights cast to fp16 (all on-chip matmuls run fp16 x fp16 -> fp32
    PSUM; plain fp32 matmuls are broken in this stack and fp32r is only
    ~tf32 anyway).

Per-core layouts (SBUF):
  xnT/cnT/qT/kT : [128, 8, 1024] fp16 (partition = dim-within-chunk,
                  free = (chunk, token))
  v_aug         : [128, 8, 16, 66] fp16 (partition = tok-within-tile,
                  free = (tok_tile, head, [ones, v(64), ones]))
  aoT           : [128, 8, 1024] fp16 (attn_out^T / 256, normalized in
                  place at the end)

Attention per head h (chunk ch=h//2, row base r0=(h%2)*64):
  s^T = K @ Q^T via lhsT=kT slice (K=64), psum fp32
  e   = exp(0.125 * s^T) * ebt[h]   (ACT exp -> fp16, DVE multiply)
  attn@v: lhsT = v_aug slice with a ones column appended so the psum
  output also carries the softmax row-sums (even head: M=65, sum in row
  64; odd head: M=128 with 63 junk columns so data lands at rows 64..127
  and the sum at row 63 -- engine/psum partition bases must be 0/32/64).
  aoT = po * 2^-8; rowsums * 2^-8 -> reciprocal -> broadcast over 128
  partitions via a tiny K=2 matmul with a 2-row selector -> multiply.
Heads are software-pipelined (attn@v emitted one head behind scores) so
the in-order PE never waits on the ACT exp chain.
"""

import numpy as np

import concourse.bass as bass
import concourse.bacc as bacc
import concourse.tile as tile
from concourse import mybir
from concourse.masks import make_identity
from concourse.bass_utils import run_bass_kernel_spmd

f32 = mybir.dt.float32
f16 = mybir.dt.float16
AF = mybir.ActivationFunctionType
ALU = mybir.AluOpType

N = 1024
D = 1024
H = 16
NT = 8
KC = 8
EPS = 1e-5
SM_SCALE = 0.125
R256 = 1.0 / 256.0
N_CORES = 8


def _ln_tile(nc, pool_a, ptmp, dst_T, src_dram, t, ident, eps_t):
    """LayerNorm (no gamma/beta) + PE transpose of token tile t."""
    xt = pool_a.tile([128, D], f32, tag="xt")
    nc.sync.dma_start(out=xt[:], in_=src_dram[t * 128:(t + 1) * 128, :])
    stats = pool_a.tile([128, 2, 6], f32, tag="stats")
    xv = xt[:].rearrange("p (a b) -> p a b", a=2)
    nc.vector.bn_stats(out=stats[:, 0, :], in_=xv[:, 0, :])
    nc.vector.bn_stats(out=stats[:, 1, :], in_=xv[:, 1, :])
    mv = pool_a.tile([128, 2], f32, tag="mv")
    nc.vector.bn_aggr(out=mv[:], in_=stats[:])
    sd = pool_a.tile([128, 2], f32, tag="sd")
    nc.scalar.activation(out=sd[:, 0:1], in_=mv[:, 1:2], func=AF.Sqrt,
                         bias=eps_t[:], scale=1.0)
    nc.vector.reciprocal(out=sd[:, 1:2], in_=sd[:, 0:1])
    xn16 = pool_a.tile([128, D], f16, tag="xn16")
    nc.vector.tensor_scalar(out=xn16[:], in0=xt[:], scalar1=mv[:, 0:1],
                            scalar2=sd[:, 1:2], op0=ALU.subtract, op1=ALU.mult)
    ptr = ptmp.tile([128, KC, 128], f16, tag="tmp")
    for c in range(KC):
        nc.tensor.transpose(ptr[:, c, :], xn16[:, c * 128:(c + 1) * 128],
                            ident[:])
    nc.vector.tensor_copy(out=dst_T[:, :, t * 128:(t + 1) * 128], in_=ptr[:])


def _body(tc, nc, x_in, c_in, wq_in, wk_in, wv_in, wo_in, bqkv_in, bo_in,
          ebt_in, sel2_in, out_d):
    with (
        tc.tile_pool(name="consts", bufs=1) as consts,
        tc.tile_pool(name="big", bufs=1) as big,
        tc.tile_pool(name="pacc", bufs=2, space="PSUM") as pacc,
        tc.tile_pool(name="ptmp", bufs=2, space="PSUM") as ptmp,
    ):
        ident = consts.tile([128, 128], f16)
        make_identity(nc, ident[:])
        eps_t = consts.tile([128, 1], f32)
        nc.vector.memset(eps_t[:], EPS)
        sel2 = consts.tile([2, 128], f16)
        nc.sync.dma_start(out=sel2[:], in_=sel2_in[:])
        bo_ap = bo_in[:]
        bo_b = consts.tile([128, D], f32)
        nc.sync.dma_start(out=bo_b[:], in_=bass.AP(
            tensor=bo_ap.tensor, offset=bo_ap.offset,
            ap=[[0, 128]] + list(bo_ap.ap)))
        bv_ap = bqkv_in[2, :]
        bv_b = consts.tile([128, D], f32)
        nc.sync.dma_start(out=bv_b[:], in_=bass.AP(
            tensor=bv_ap.tensor, offset=bv_ap.offset,
            ap=[[0, 128]] + list(bv_ap.ap)))
        bq_t = consts.tile([128, KC], f32)
        nc.sync.dma_start(out=bq_t[:], in_=bqkv_in[0, :].rearrange(
            "(m p) -> p m", p=128))
        bk_t = consts.tile([128, KC], f32)
        nc.sync.dma_start(out=bk_t[:], in_=bqkv_in[1, :].rearrange(
            "(m p) -> p m", p=128))

        qT = big.tile([128, KC, N], f16)
        kT = big.tile([128, KC, N], f16)
        v_aug = big.tile([128, NT, H, 66], f16)
        aoT = big.tile([128, KC, N], f16)
        nc.gpsimd.memset(v_aug[:, :, :, 0:1], 1.0)
        nc.gpsimd.memset(v_aug[:, :, :, 65:66], 1.0)
        v_flat = v_aug[:].rearrange("p a h c -> p a (h c)")

        with tc.tile_pool(name="pact", bufs=1) as pact, \
             tc.tile_pool(name="pa", bufs=3) as pool_a:
            with tc.tile_pool(name="pw", bufs=1) as pw:
                wq16 = pw.tile([128, KC, D], f16)
                nc.sync.dma_start(out=wq16[:], in_=wq_in.rearrange(
                    "(a p) m -> p a m", p=128))
                wk16 = pw.tile([128, KC, D], f16)
                nc.sync.dma_start(out=wk16[:], in_=wk_in.rearrange(
                    "(a p) m -> p a m", p=128))
                wv16 = pw.tile([128, KC, D], f16)
                nc.sync.dma_start(out=wv16[:], in_=wv_in.rearrange(
                    "(a p) m -> p a m", p=128))

                xnT = pact.tile([128, KC, N], f16, tag="xnT")
                cnT = pact.tile([128, KC, N], f16, tag="cnT")

                def emit_proj(srcT, w16, b_t, dstT):
                    for m in range(KC):
                        pq = pacc.tile([128, N], f32, tag="acc")
                        for kc in range(KC):
                            for nh in range(2):
                                nc.tensor.matmul(
                                    pq[:, nh * 512:(nh + 1) * 512],
                                    w16[:, kc, m * 128:(m + 1) * 128],
                                    srcT[:, kc, nh * 512:(nh + 1) * 512],
                                    start=(kc == 0), stop=(kc == KC - 1))
                        nc.scalar.add(out=dstT[:, m, :], in_=pq[:],
                                      add=b_t[:, m:m + 1])

                for t in range(NT):
                    _ln_tile(nc, pool_a, ptmp, xnT, x_in, t, ident, eps_t)
                emit_proj(xnT, wq16, bq_t, qT)
                for t in range(NT):
                    _ln_tile(nc, pool_a, ptmp, cnT, c_in, t, ident, eps_t)
                emit_proj(cnT, wk16, bk_t, kT)

                for t in range(NT):
                    pv = pacc.tile([128, N], f32, tag="acc")
                    for kc in range(KC):
                        for nh in range(2):
                            nc.tensor.matmul(
                                pv[:, nh * 512:(nh + 1) * 512],
                                cnT[:, kc, t * 128:(t + 1) * 128],
                                wv16[:, kc, nh * 512:(nh + 1) * 512],
                                start=(kc == 0), stop=(kc == KC - 1))
                    nc.vector.tensor_add(
                        out=v_aug[:, t, :, 1:65],
                        in0=pv[:].rearrange("p (h d) -> p h d", h=H),
                        in1=bv_b[:].rearrange("p (h d) -> p h d", h=H))

        with tc.tile_pool(name="pwo", bufs=1) as pwo, \
             tc.tile_pool(name="pc", bufs=2) as pc, \
             tc.tile_pool(name="prs", bufs=2) as prs:
            wo16 = pwo.tile([128, KC, D], f16, tag="wo")
            nc.sync.dma_start(out=wo16[:], in_=wo_in.rearrange(
                "(a p) m -> p a m", p=128))

            def emit_scores(h, eh, ebh):
                ch, r0 = h // 2, (h % 2) * 64
                for kt in range(NT):
                    ps_s = ptmp.tile([128, N], f32, tag="tmp")
                    for nh in range(2):
                        nc.tensor.matmul(
                            ps_s[:, nh * 512:(nh + 1) * 512],
                            kT[r0:r0 + 64, ch, kt * 128:(kt + 1) * 128],
                            qT[r0:r0 + 64, ch, nh * 512:(nh + 1) * 512],
                            start=True, stop=True)
                    nc.scalar.activation(out=eh[:, kt, :], in_=ps_s[:],
                                         func=AF.Exp, scale=SM_SCALE)
                    nc.vector.tensor_mul(out=eh[:, kt, :], in0=eh[:, kt, :],
                                         in1=ebh[:, kt, :])

            def emit_attnv(h, eh, rrec_all, rr_pair):
                ch = h // 2
                po = pacc.tile([128, N], f32, tag="acc")
                if h % 2 == 0:
                    lo, ssum, sdat, dst = h * 66 + 1, 64, 0, aoT[0:64, ch, :]
                    mm = 65
                else:
                    lo, ssum, sdat, dst = h * 66 - 63, 63, 64, aoT[64:128, ch, :]
                    mm = 128
                for kt in range(NT):
                    for nh in range(2):
                        nc.tensor.matmul(
                            po[0:mm, nh * 512:(nh + 1) * 512],
                            v_flat[:, kt, lo:lo + mm],
                            eh[:, kt, nh * 512:(nh + 1) * 512],
                            start=(kt == 0), stop=(kt == NT - 1))
                nc.scalar.mul(out=dst, in_=po[sdat:sdat + 64, :], mul=R256)
                rs_st = prs.tile([66, N], f32, tag="rsst")
                cb, cn_ = (64, 1) if h % 2 == 0 else (32, 32)
                nc.scalar.mul(out=rs_st[cb:cb + cn_, :],
                              in_=po[cb:cb + cn_, :], mul=R256)
                nc.sync.dma_start(out=rr_pair[h % 2:h % 2 + 1, :],
                                  in_=rs_st[ssum:ssum + 1, :])
                if h % 2 == 1:
                    with nc.allow_low_precision(reason="recip of rowsum"):
                        nc.vector.reciprocal(out=rrec_all[:, ch, :],
                                             in_=rr_pair[:])

            def emit_normalize(ch):
                p_rt = ptmp.tile([128, N], f32, tag="tmp")
                for nh in range(2):
                    nc.tensor.matmul(
                        p_rt[:, nh * 512:(nh + 1) * 512], sel2[:],
                        rrec_all[:, ch, nh * 512:(nh + 1) * 512],
                        start=True, stop=True)
                nc.vector.tensor_mul(out=aoT[:, ch, :], in0=aoT[:, ch, :],
                                     in1=p_rt[:])

            rrec_all = pwo.tile([2, KC, N], f16, tag="rrec")
            prev = None
            rr_pair = None
            for h in range(H):
                ebh = pc.tile([128, NT, N], f16, tag="ebt")
                nc.sync.dma_start(out=ebh[:], in_=ebt_in[h].rearrange(
                    "(a p) q -> p a q", p=128))
                eh = pc.tile([128, NT, N], f16, tag="et")
                emit_scores(h, eh, ebh)
                if h % 2 == 0:
                    rr_new = prs.tile([2, N], f32, tag="rrin")
                else:
                    rr_new = rr_pair
                if prev is not None:
                    emit_attnv(prev[0], prev[1], rrec_all, prev[2])
                prev = (h, eh, rr_new)
                rr_pair = rr_new
                if h >= 3 and h % 2 == 1:
                    emit_normalize((h - 3) // 2)
            emit_attnv(prev[0], prev[1], rrec_all, prev[2])
            emit_normalize(KC - 1)

            for m in range(NT):
                fo = pacc.tile([128, N], f32, tag="acc")
                for kc in range(KC):
                    for nh in range(2):
                        nc.tensor.matmul(
                            fo[:, nh * 512:(nh + 1) * 512],
                            aoT[:, kc, m * 128:(m + 1) * 128],
                            wo16[:, kc, nh * 512:(nh + 1) * 512],
                            start=(kc == 0), stop=(kc == KC - 1))
                so = pwo.tile([128, N], f32, tag="so")
                nc.vector.tensor_add(out=so[:], in0=fo[:], in1=bo_b[:])
                nc.sync.dma_start(out=out_d[m * 128:(m + 1) * 128, :], in_=so[:])


def build():
    nc = bacc.Bacc()
    x_in = nc.declare_dram_parameter("x", [N, D], f32, isOutput=False)
    c_in = nc.declare_dram_parameter("ctx", [N, D], f32, isOutput=False)
    wq_in = nc.declare_dram_parameter("wq", [D, D], f16, isOutput=False)
    wk_in = nc.declare_dram_parameter("wk", [D, D], f16, isOutput=False)
    wv_in = nc.declare_dram_parameter("wv", [D, D], f16, isOutput=False)
    wo_in = nc.declare_dram_parameter("wo", [D, D], f16, isOutput=False)
    bqkv_in = nc.declare_dram_parameter("bqkv", [3, D], f32, isOutput=False)
    bo_in = nc.declare_dram_parameter("bo", [D], f32, isOutput=False)
    ebt_in = nc.declare_dram_parameter("ebt", [H, N, N], f16, isOutput=False)
    sel2_in = nc.declare_dram_parameter("sel2", [2, 128], f16, isOutput=False)
    out_d = nc.declare_dram_parameter("out", [N, D], f32, isOutput=True)
    with tile.TileContext(nc) as tc:
        _body(tc, nc, x_in, c_in, wq_in, wk_in, wv_in, wo_in, bqkv_in, bo_in,
              ebt_in, sel2_in, out_d)
    nc.compile()
    return nc


_NC_CACHE = None


def _get_nc():
    global _NC_CACHE
    if _NC_CACHE is None:
        _NC_CACHE = build()
    return _NC_CACHE


def kernel(x, context, relative_position_bias, Wq, Wk, Wv, Wo, bo, gamma,
           beta):
    x = np.asarray(x, np.float32)
    context = np.asarray(context, np.float32)
    rpb = np.asarray(relative_position_bias, np.float32)
    Wq = np.asarray(Wq, np.float32)
    Wk = np.asarray(Wk, np.float32)
    Wv = np.asarray(Wv, np.float32)
    Wo = np.asarray(Wo, np.float32)
    bo = np.asarray(bo, np.float32)
    gamma = np.asarray(gamma, np.float32)
    beta = np.asarray(beta, np.float32)

    wq16 = (gamma[:, None] * Wq).astype(np.float16)
    wk16 = (gamma[:, None] * Wk).astype(np.float16)
    wv16 = (gamma[:, None] * Wv).astype(np.float16)
    wo16 = Wo.astype(np.float16)
    bqkv = np.stack([beta @ Wq, beta @ Wk, beta @ Wv]).astype(np.float32)
    ebt = np.exp(rpb).transpose(0, 2, 1).astype(np.float16).copy()
    sel2 = np.zeros((2, 128), np.float16)
    sel2[0, 0:64] = 1.0
    sel2[1, 64:128] = 1.0

    shared = {
        "wq": wq16, "wk": wk16, "wv": wv16, "wo": wo16,
        "bqkv": bqkv, "bo": bo, "ebt": ebt, "sel2": sel2,
    }
    in_maps = [
        {"x": np.ascontiguousarray(x[i]),
         "ctx": np.ascontiguousarray(context[i]), **shared}
        for i in range(N_CORES)
    ]

    nc = _get_nc()
    res = run_bass_kernel_spmd(nc, in_maps, list(range(N_CORES)))
    return np.stack([res.results[i]["out"] for i in range(N_CORES)])
